# revision 11
# baseline (speedup 1.0000x reference)
"""Bahdanau-attention Trainium2 kernel (8 NeuronCores, data-parallel over batch).

Problem (S=2048, B=16, H=1024):
    energy  = tanh(cat([hidden, enc], -1) @ W.T + b)      [S,B,H]
    scores  = energy . v                                   [B,S]
    attn    = softmax(scores, axis=S)                      [B,S]
    context = sum_s attn[s] * enc[s]                       [B,H]
    returns (context, attn)

Sharding: batch B=16 split 2-per-core across 8 cores; W/b/v replicated.
No collectives needed.

Per-core device program (batch-local b in {0,1}):
  - main matmul: energy_pre[s,h] = sum_k E[s,k]*W2T[k,h] (+ qb[h] via a
    PSUM-init matmul against a one-hot row), tiles out[m=s:128, n=h:512],
    with E^T (k-major) as lhsT and W2^T as moving rhs, fp32r pipeline.
  - tanh on ScalarE (PSUM -> SBUF).
  - scores.T on DVE: fused tensor_tensor_reduce with v broadcast,
    accumulated into [128, 16] (s on partitions).
  - softmax in [128,16] layout; cross-partition max/sum via PE transpose
    + one-hot-row broadcast matmuls.
  - context[h] = sum_s attn[s]*E[s,h] on PE: attn column stationary,
    natural-layout E tiles moving.

The hidden-state projection qb = hidden @ W1.T + b (0.4% of FLOPs) is
computed on host and shipped as a bias row.
"""

from contextlib import ExitStack

import numpy as np

S, B, H = 2048, 16, 1024
P = 128
NCORES = 8
BL = B // NCORES          # batches per core = 2
KO = H // P               # 8 contraction chunks
ST = S // P               # 16 s-tiles per batch
NF = 512                  # matmul moving free dim
NCH = H // NF             # 2 h_out chunks

_CACHE = {}


def _build(mm_dtype: str = "f32r", repeat: int = 1, debug_mode: str = "full"):
    # debug_mode: "main" = energy+scores only; "nosm" = +softmax (no context,
    # contiguous attn dma); "nostride" = full but contiguous attn dma; "full".
    import concourse.mybir as mybir
    import concourse.tile as tile
    from concourse import bacc

    F32 = mybir.dt.float32
    F32R = mybir.dt.float32r
    AF = mybir.ActivationFunctionType

    nc = bacc.Bacc("TRN2", target_bir_lowering=False, debug=False, num_devices=NCORES)

    # fp32r tensors carry plain fp32 bytes; declaring them f32r end-to-end
    # satisfies walrus's "rounded to FP32r" producer check.
    mm_dt = {"f32r": F32R, "f32": F32}[mm_dtype]

    e_t = nc.dram_tensor("e_t", [BL, H, S], mm_dt, kind="ExternalInput").ap()
    e_nat = nc.dram_tensor("e_nat", [BL, S, H], mm_dt, kind="ExternalInput").ap()
    w2t = nc.dram_tensor("w2t", [H, H], mm_dt, kind="ExternalInput").ap()
    qbp = nc.dram_tensor("qbp", [P, BL * H], mm_dt, kind="ExternalInput").ap()
    vb = nc.dram_tensor("vb", [P, H], F32, kind="ExternalInput").ap()
    bc1 = nc.dram_tensor("bc1", [P, P], mm_dt, kind="ExternalInput").ap()
    bc1f = nc.dram_tensor("bc1f", [P, P], F32, kind="ExternalInput").ap()
    ident = nc.dram_tensor("ident", [P, P], F32, kind="ExternalInput").ap()
    ctx_out = nc.dram_tensor("ctx_out", [BL, H], F32, kind="ExternalOutput").ap()
    attn_out = nc.dram_tensor("attn_out", [BL, S], F32, kind="ExternalOutput").ap()

    with tile.TileContext(nc) as tc, ExitStack() as ctx:
        const = ctx.enter_context(tc.tile_pool(name="const", bufs=1))
        etp = ctx.enter_context(tc.tile_pool(name="etp", bufs=2))
        enp = ctx.enter_context(tc.tile_pool(name="enp", bufs=1))
        enrg = ctx.enter_context(tc.tile_pool(name="enrg", bufs=3))
        trashp = ctx.enter_context(tc.tile_pool(name="trashp", bufs=2))
        smp = ctx.enter_context(tc.tile_pool(name="smp", bufs=2))
        ctxp = ctx.enter_context(tc.tile_pool(name="ctxp", bufs=1))
        eps = ctx.enter_context(tc.tile_pool(name="eps", bufs=3, space="PSUM"))
        sps = ctx.enter_context(tc.tile_pool(name="sps", bufs=1, space="PSUM"))
        cps = ctx.enter_context(tc.tile_pool(name="cps", bufs=2, space="PSUM"))

        # ---- persistent constants ----
        w2t_sb = const.tile([P, KO, H], mm_dt)
        nc.sync.dma_start(w2t_sb[:], w2t.rearrange("(ko p) n -> p ko n", p=P))
        vb_sb = const.tile([P, H], F32)
        nc.sync.dma_start(vb_sb[:], vb)
        qb_sb = const.tile([P, BL * H], mm_dt)
        nc.sync.dma_start(qb_sb[:], qbp)
        bc1_sb = const.tile([P, P], mm_dt)
        nc.sync.dma_start(bc1_sb[:], bc1)
        bc1f_sb = const.tile([P, P], F32)
        nc.sync.dma_start(bc1f_sb[:], bc1f)
        ident_sb = const.tile([P, P], F32)
        nc.sync.dma_start(ident_sb[:], ident)

        for b in [b for _ in range(repeat) for b in range(BL)]:
            # prefetch natural-layout E for the context matvec
            en_sb = enp.tile([P, ST, H], mm_dt, tag="e_nat")
            nc.sync.dma_start(en_sb[:], e_nat[b].rearrange("(t p) h -> p t h", p=P))

            scores_sb = smp.tile([P, ST], F32, tag="scores")

            for sig in range(S // NF):  # 4 chunks of 512 s-positions
                et_sb = etp.tile([P, KO, NF], mm_dt, tag="et")
                nc.sync.dma_start(
                    et_sb[:],
                    e_t[b].rearrange("(ko p) s -> p ko s", p=P)[
                        :, :, sig * NF : (sig + 1) * NF
                    ],
                )
                for tl in range(NF // P):  # 4 s-tiles of 128
                    t_idx = sig * (NF // P) + tl
                    energy = enrg.tile([P, H], F32, tag="energy")
                    for n in range(NCH):
                        ps = eps.tile([P, NF], F32, tag="ps")
                        # PSUM init with qb[h] broadcast over partitions:
                        # bc1 has only row k=0 set to 1 -> out[m,n]=qbp[0,n].
                        nc.tensor.matmul(
                            ps[:],
                            bc1_sb[:],
                            qb_sb[:, b * H + n * NF : b * H + (n + 1) * NF],
                            start=True,
                            stop=False,
                        )
                        for k in range(KO):
                            nc.tensor.matmul(
                                ps[:],
                                et_sb[:, k, tl * P : (tl + 1) * P],
                                w2t_sb[:, k, n * NF : (n + 1) * NF],
                                start=False,
                                stop=(k == KO - 1),
                            )
                        nc.scalar.activation(
                            energy[:, n * NF : (n + 1) * NF], ps[:], AF.Tanh
                        )
                    if debug_mode == "mm":
                        continue
                    trash = trashp.tile([P, H], F32, tag="trash")
                    nc.vector.tensor_tensor(
                        trash[:], energy[:], vb_sb[:], mybir.AluOpType.mult
                    )
                    nc.vector.tensor_reduce(
                        scores_sb[:, t_idx : t_idx + 1],
                        trash[:],
                        axis=mybir.AxisListType.X,
                        op=mybir.AluOpType.add,
                    )

            if debug_mode in ("main", "mm"):
                if debug_mode == "main":
                    nc.sync.dma_start(
                        attn_out[b].rearrange("(p o) -> p o", p=P), scores_sb[:]
                    )
                dbg_ctx = ctxp.tile([1, H], F32, tag=f"ctx{b}")
                nc.vector.tensor_copy(dbg_ctx[:], energy[0:1, :])
                nc.sync.dma_start(ctx_out[b : b + 1, :], dbg_ctx[:])
                continue

            # ---- softmax over s (partitions x columns of [128,16]) ----
            mx = smp.tile([P, 1], F32, tag="mx")
            nc.vector.tensor_reduce(
                mx[:], scores_sb[:], axis=mybir.AxisListType.X, op=mybir.AluOpType.max
            )
            ps_t = sps.tile([1, P], F32, tag="pst")
            nc.tensor.transpose(ps_t[:], mx[:], ident_sb[:])
            row = smp.tile([1, P], F32, tag="row")
            nc.scalar.copy(row[:], ps_t[:])
            gmax = smp.tile([1, 1], F32, tag="gmax")
            nc.vector.tensor_reduce(
                gmax[:], row[:], axis=mybir.AxisListType.X, op=mybir.AluOpType.max
            )
            nzcol = smp.tile([P, 1], F32, tag="nzcol")
            nc.vector.memset(nzcol[:], 0.0)
            nc.vector.tensor_scalar_mul(nzcol[0:1, :], gmax[:], -1.0)
            ps_b = sps.tile([P, 1], F32, tag="psb")
            nc.tensor.matmul(ps_b[:], bc1f_sb[:], nzcol[:], start=True, stop=True)
            nmax_col = smp.tile([P, 1], F32, tag="nmax")
            nc.scalar.copy(nmax_col[:], ps_b[:])

            expv = smp.tile([P, ST], F32, tag="expv")
            sume = smp.tile([P, 1], F32, tag="sume")
            nc.scalar.activation(
                expv[:], scores_sb[:], AF.Exp, bias=nmax_col[:], accum_out=sume[:]
            )
            ps_t2 = sps.tile([1, P], F32, tag="pst")
            nc.tensor.transpose(ps_t2[:], sume[:], ident_sb[:])
            row2 = smp.tile([1, P], F32, tag="row")
            nc.scalar.copy(row2[:], ps_t2[:])
            z = smp.tile([1, 1], F32, tag="z")
            nc.vector.tensor_reduce(
                z[:], row2[:], axis=mybir.AxisListType.X, op=mybir.AluOpType.add
            )
            rz = smp.tile([1, 1], F32, tag="rz")
            nc.vector.reciprocal(rz[:], z[:])
            zcol = smp.tile([P, 1], F32, tag="zcol")
            nc.vector.memset(zcol[:], 0.0)
            nc.vector.tensor_copy(zcol[0:1, :], rz[:])
            ps_b2 = sps.tile([P, 1], F32, tag="psb")
            nc.tensor.matmul(ps_b2[:], bc1f_sb[:], zcol[:], start=True, stop=True)
            rz_col = smp.tile([P, 1], F32, tag="rzcol")
            nc.scalar.copy(rz_col[:], ps_b2[:])

            attn_sb = smp.tile([P, ST], mm_dt, tag="attn")
            nc.vector.tensor_scalar_mul(attn_sb[:], expv[:], rz_col[:])
            if debug_mode in ("nosm", "nostride"):
                nc.sync.dma_start(
                    attn_out[b].rearrange("(p o) -> p o", p=P),
                    attn_sb[:].bitcast(mybir.dt.float32),
                )
            else:
                nc.sync.dma_start(
                    attn_out[b].rearrange("(o p) -> p o", p=P),
                    attn_sb[:].bitcast(mybir.dt.float32),
                )
            if debug_mode == "nosm":
                dbg_ctx = ctxp.tile([1, H], F32, tag=f"ctx{b}")
                nc.vector.tensor_copy(dbg_ctx[:], energy[0:1, :])
                nc.sync.dma_start(ctx_out[b : b + 1, :], dbg_ctx[:])
                continue

            # ---- context[h] = sum_s attn[s] * E[s,h] ----
            ctx_sb = ctxp.tile([1, H], F32, tag=f"ctx{b}")
            for n in range(NCH):
                pc = cps.tile([1, NF], F32, tag="pc")
                for t in range(ST):
                    nc.tensor.matmul(
                        pc[:],
                        attn_sb[:, t : t + 1],
                        en_sb[:, t, n * NF : (n + 1) * NF],
                        start=(t == 0),
                        stop=(t == ST - 1),
                    )
                nc.scalar.copy(ctx_sb[:, n * NF : (n + 1) * NF], pc[:])
            nc.sync.dma_start(ctx_out[b : b + 1, :], ctx_sb[:])

    nc.compile()
    return nc


def _get_nc():
    if "nc" not in _CACHE:
        _CACHE["nc"] = _build()
    return _CACHE["nc"]


def _make_runner(nc):
    """Build a cached jitted SPMD runner for `nc` (one NEFF compile total).

    Mirrors concourse.bass2jax.run_bass_via_pjrt, but keeps the jitted
    callable so repeat invocations don't re-trace/re-compile.
    """
    import jax
    import numpy as jnp_np  # noqa: F401
    from jax.sharding import Mesh, PartitionSpec
    from jax.experimental.shard_map import shard_map

    import concourse.mybir as mybir
    from concourse import bass2jax

    bass2jax.install_neuronx_cc_hook()

    partition_name = nc.partition_id_tensor.name if nc.partition_id_tensor else None

    in_names, out_names, out_avals, zero_outs = [], [], [], []
    for alloc in nc.m.functions[0].allocations:
        if not isinstance(alloc, mybir.MemoryLocationSet):
            continue
        name = alloc.memorylocations[0].name
        if alloc.kind == "ExternalInput":
            if name != partition_name:
                in_names.append(name)
        elif alloc.kind == "ExternalOutput":
            out_names.append(name)
            shape = tuple(alloc.tensor_shape)
            dtype = mybir.dt.np(alloc.dtype)
            out_avals.append(jax.core.ShapedArray(shape, dtype))
            zero_outs.append(np.zeros(shape, dtype))
    n_params = len(in_names)
    all_names = in_names + out_names
    if partition_name is not None:
        all_names = all_names + [partition_name]

    def _body(*args):
        operands = list(args)
        if partition_name is not None:
            operands.append(bass2jax.partition_id_tensor())
        outs = bass2jax._bass_exec_p.bind(
            *operands,
            out_avals=tuple(out_avals),
            in_names=tuple(all_names),
            out_names=tuple(out_names),
            lowering_input_output_aliases=(),
            sim_require_finite=True,
            sim_require_nnan=True,
            nc=nc,
        )
        return tuple(outs)

    devices = jax.devices()[:NCORES]
    mesh = Mesh(np.asarray(devices), ("core",))
    n_args = n_params + len(out_names)
    sharded = jax.jit(
        shard_map(
            _body,
            mesh=mesh,
            in_specs=(PartitionSpec("core"),) * n_args,
            out_specs=(PartitionSpec("core"),) * len(out_names),
            check_rep=False,
        ),
        keep_unused=True,
    )

    def run(in_maps):
        concat_in = [
            np.concatenate([np.asarray(m[name]) for m in in_maps], axis=0)
            for name in in_names
        ]
        concat_zeros = [
            np.zeros((NCORES * z.shape[0], *z.shape[1:]), z.dtype) for z in zero_outs
        ]
        out_arrs = sharded(*concat_in, *concat_zeros)
        return [
            {
                name: np.asarray(out_arrs[i]).reshape(
                    NCORES, *out_avals[i].shape
                )[c]
                for i, name in enumerate(out_names)
            }
            for c in range(NCORES)
        ]

    run.sharded = sharded
    run.in_names = in_names
    run.out_names = out_names
    run.out_avals = out_avals
    run.zero_outs = zero_outs
    run.mesh = mesh
    return run


def _get_runner():
    if "runner" not in _CACHE:
        _CACHE["runner"] = _make_runner(_get_nc())
    return _CACHE["runner"]


def _prep_inputs(hidden, encoder_outputs, W, b, v):
    hidden = np.asarray(hidden, dtype=np.float32)
    enc = np.asarray(encoder_outputs, dtype=np.float32)
    W = np.asarray(W, dtype=np.float32)
    b = np.asarray(b, dtype=np.float32)
    v = np.asarray(v, dtype=np.float32)

    qb_all = (hidden[0] @ W[:, :H].T + b).astype(np.float32)  # [B, H]
    w2t_h = np.ascontiguousarray(W[:, H:].T)                  # [H, H]
    vb_h = np.ascontiguousarray(np.broadcast_to(v, (P, H)))
    bc1_h = np.zeros((P, P), np.float32)
    bc1_h[0, :] = 1.0
    ident_h = np.eye(P, dtype=np.float32)

    in_maps = []
    for c in range(NCORES):
        bs = slice(BL * c, BL * (c + 1))
        e_b = enc[:, bs, :]  # [S, BL, H]
        e_nat = np.ascontiguousarray(e_b.transpose(1, 0, 2))  # [BL, S, H]
        e_tt = np.ascontiguousarray(e_b.transpose(1, 2, 0))   # [BL, H, S]
        qbp = np.zeros((P, BL * H), np.float32)
        qbp[0, :] = qb_all[bs].reshape(-1)
        in_maps.append(
            dict(
                e_t=e_tt,
                e_nat=e_nat,
                w2t=w2t_h,
                qbp=qbp,
                vb=vb_h,
                bc1=bc1_h,
                bc1f=bc1_h,
                ident=ident_h,
            )
        )
    return in_maps


def kernel(hidden, encoder_outputs, W, b, v):
    in_maps = _prep_inputs(hidden, encoder_outputs, W, b, v)
    results = _get_runner()(in_maps)
    context = np.concatenate(
        [results[c]["ctx_out"] for c in range(NCORES)], axis=0
    ).astype(np.float32)
    attn = np.concatenate(
        [results[c]["attn_out"] for c in range(NCORES)], axis=0
    ).astype(np.float32)
    return context, attn


# revision 12
# speedup vs baseline: 2.5991x; 2.5991x over previous
"""Bahdanau-attention Trainium2 kernel (8 NeuronCores, data-parallel over batch).

Problem (S=2048, B=16, H=1024):
    energy  = tanh(cat([hidden, enc], -1) @ W.T + b)      [S,B,H]
    scores  = energy . v                                   [B,S]
    attn    = softmax(scores, axis=S)                      [B,S]
    context = sum_s attn[s] * enc[s]                       [B,H]
    returns (context, attn)

Sharding: batch B=16 split 2-per-core across 8 cores; W/b/v replicated.
No collectives needed.

Per-core device program (batch-local b in {0,1}):
  - main matmul: energy_pre[s,h] = sum_k E[s,k]*W2T[k,h] (+ qb[h] via a
    PSUM-init matmul against a one-hot row), tiles out[m=s:128, n=h:512],
    with E^T (k-major) as lhsT and W2^T as moving rhs, fp32r pipeline.
  - tanh on ScalarE (PSUM -> SBUF).
  - scores.T on DVE: fused tensor_tensor_reduce with v broadcast,
    accumulated into [128, 16] (s on partitions).
  - softmax in [128,16] layout; cross-partition max/sum via PE transpose
    + one-hot-row broadcast matmuls.
  - context[h] = sum_s attn[s]*E[s,h] on PE: attn column stationary,
    natural-layout E tiles moving.

The hidden-state projection qb = hidden @ W1.T + b (0.4% of FLOPs) is
computed on host and shipped as a bias row.
"""

from contextlib import ExitStack

import numpy as np

S, B, H = 2048, 16, 1024
P = 128
NCORES = 8
BL = B // NCORES          # batches per core = 2
KO = H // P               # 8 contraction chunks
ST = S // P               # 16 s-tiles per batch
NF = 512                  # matmul moving free dim
NCH = H // NF             # 2 h_out chunks

_CACHE = {}


def _build(mm_dtype: str = "f32r", repeat: int = 1, debug_mode: str = "full",
           loop_n: int = 1):
    # debug_mode: "main" = energy+scores only; "nosm" = +softmax (no context,
    # contiguous attn dma); "nostride" = full but contiguous attn dma; "full".
    import concourse.mybir as mybir
    import concourse.tile as tile
    from concourse import bacc

    F32 = mybir.dt.float32
    F32R = mybir.dt.float32r
    AF = mybir.ActivationFunctionType

    nc = bacc.Bacc("TRN2", target_bir_lowering=False, debug=False, num_devices=NCORES)

    # fp32r tensors carry plain fp32 bytes; declaring them f32r end-to-end
    # satisfies walrus's "rounded to FP32r" producer check.
    mm_dt = {"f32r": F32R, "f32": F32}[mm_dtype]

    e_t = nc.dram_tensor("e_t", [BL, H, S], mm_dt, kind="ExternalInput").ap()
    e_nat = nc.dram_tensor("e_nat", [BL, S, H], mm_dt, kind="ExternalInput").ap()
    w2t = nc.dram_tensor("w2t", [H, H], mm_dt, kind="ExternalInput").ap()
    qbp = nc.dram_tensor("qbp", [P, BL * H], mm_dt, kind="ExternalInput").ap()
    vb = nc.dram_tensor("vb", [P, H], F32, kind="ExternalInput").ap()
    bc1 = nc.dram_tensor("bc1", [P, P], mm_dt, kind="ExternalInput").ap()
    bc1f = nc.dram_tensor("bc1f", [P, P], F32, kind="ExternalInput").ap()
    ident = nc.dram_tensor("ident", [P, P], F32, kind="ExternalInput").ap()
    ctx_out = nc.dram_tensor("ctx_out", [BL, H], F32, kind="ExternalOutput").ap()
    attn_out = nc.dram_tensor("attn_out", [BL, S], F32, kind="ExternalOutput").ap()

    with tile.TileContext(nc) as tc, ExitStack() as ctx:
        const = ctx.enter_context(tc.tile_pool(name="const", bufs=1))
        etp = ctx.enter_context(tc.tile_pool(name="etp", bufs=2))
        enp = ctx.enter_context(tc.tile_pool(name="enp", bufs=1))
        enrg = ctx.enter_context(tc.tile_pool(name="enrg", bufs=3))
        trashp = ctx.enter_context(tc.tile_pool(name="trashp", bufs=2))
        smp = ctx.enter_context(tc.tile_pool(name="smp", bufs=2))
        ctxp = ctx.enter_context(tc.tile_pool(name="ctxp", bufs=1))
        eps = ctx.enter_context(tc.tile_pool(name="eps", bufs=3, space="PSUM"))
        sps = ctx.enter_context(tc.tile_pool(name="sps", bufs=1, space="PSUM"))
        cps = ctx.enter_context(tc.tile_pool(name="cps", bufs=2, space="PSUM"))

        # ---- persistent constants ----
        w2t_sb = const.tile([P, KO, H], mm_dt)
        nc.sync.dma_start(w2t_sb[:], w2t.rearrange("(ko p) n -> p ko n", p=P))
        vb_sb = const.tile([P, H], F32)
        nc.sync.dma_start(vb_sb[:], vb)
        qb_sb = const.tile([P, BL * H], mm_dt)
        nc.sync.dma_start(qb_sb[:], qbp)
        bc1_sb = const.tile([P, P], mm_dt)
        nc.sync.dma_start(bc1_sb[:], bc1)
        bc1f_sb = const.tile([P, P], F32)
        nc.sync.dma_start(bc1f_sb[:], bc1f)
        ident_sb = const.tile([P, P], F32)
        nc.sync.dma_start(ident_sb[:], ident)

        loop_cm = tc.For_i(0, loop_n, 1) if loop_n > 1 else None
        if loop_cm is not None:
            loop_cm.__enter__()
        for b in [b for _ in range(repeat) for b in range(BL)]:
            # prefetch natural-layout E for the context matvec
            en_sb = enp.tile([P, ST, H], mm_dt, tag="e_nat")
            nc.sync.dma_start(en_sb[:], e_nat[b].rearrange("(t p) h -> p t h", p=P))

            scores_sb = smp.tile([P, ST], F32, tag="scores")

            for sig in range(S // NF):  # 4 chunks of 512 s-positions
                et_sb = etp.tile([P, KO, NF], mm_dt, tag="et")
                nc.sync.dma_start(
                    et_sb[:],
                    e_t[b].rearrange("(ko p) s -> p ko s", p=P)[
                        :, :, sig * NF : (sig + 1) * NF
                    ],
                )
                for tl in range(NF // P):  # 4 s-tiles of 128
                    t_idx = sig * (NF // P) + tl
                    energy = enrg.tile([P, H], F32, tag="energy")
                    for n in range(NCH):
                        ps = eps.tile([P, NF], F32, tag="ps")
                        # PSUM init with qb[h] broadcast over partitions:
                        # bc1 has only row k=0 set to 1 -> out[m,n]=qbp[0,n].
                        nc.tensor.matmul(
                            ps[:],
                            bc1_sb[:],
                            qb_sb[:, b * H + n * NF : b * H + (n + 1) * NF],
                            start=True,
                            stop=False,
                        )
                        for k in range(KO):
                            nc.tensor.matmul(
                                ps[:],
                                et_sb[:, k, tl * P : (tl + 1) * P],
                                w2t_sb[:, k, n * NF : (n + 1) * NF],
                                start=False,
                                stop=(k == KO - 1),
                            )
                        nc.scalar.activation(
                            energy[:, n * NF : (n + 1) * NF], ps[:], AF.Tanh
                        )
                    if debug_mode == "mm":
                        continue
                    trash = trashp.tile([P, H], F32, tag="trash")
                    nc.vector.tensor_tensor(
                        trash[:], energy[:], vb_sb[:], mybir.AluOpType.mult
                    )
                    nc.vector.tensor_reduce(
                        scores_sb[:, t_idx : t_idx + 1],
                        trash[:],
                        axis=mybir.AxisListType.X,
                        op=mybir.AluOpType.add,
                    )

            if debug_mode in ("main", "mm"):
                if debug_mode == "main":
                    nc.sync.dma_start(
                        attn_out[b].rearrange("(p o) -> p o", p=P), scores_sb[:]
                    )
                dbg_ctx = ctxp.tile([1, H], F32, tag=f"ctx{b}")
                nc.vector.tensor_copy(dbg_ctx[:], energy[0:1, :])
                nc.sync.dma_start(ctx_out[b : b + 1, :], dbg_ctx[:])
                continue

            # ---- softmax over s (partitions x columns of [128,16]) ----
            mx = smp.tile([P, 1], F32, tag="mx")
            nc.vector.tensor_reduce(
                mx[:], scores_sb[:], axis=mybir.AxisListType.X, op=mybir.AluOpType.max
            )
            ps_t = sps.tile([1, P], F32, tag="pst")
            nc.tensor.transpose(ps_t[:], mx[:], ident_sb[:])
            row = smp.tile([1, P], F32, tag="row")
            nc.scalar.copy(row[:], ps_t[:])
            gmax = smp.tile([1, 1], F32, tag="gmax")
            nc.vector.tensor_reduce(
                gmax[:], row[:], axis=mybir.AxisListType.X, op=mybir.AluOpType.max
            )
            nzcol = smp.tile([P, 1], F32, tag="nzcol")
            nc.vector.memset(nzcol[:], 0.0)
            nc.vector.tensor_scalar_mul(nzcol[0:1, :], gmax[:], -1.0)
            ps_b = sps.tile([P, 1], F32, tag="psb")
            nc.tensor.matmul(ps_b[:], bc1f_sb[:], nzcol[:], start=True, stop=True)
            nmax_col = smp.tile([P, 1], F32, tag="nmax")
            nc.scalar.copy(nmax_col[:], ps_b[:])

            expv = smp.tile([P, ST], F32, tag="expv")
            sume = smp.tile([P, 1], F32, tag="sume")
            nc.scalar.activation(
                expv[:], scores_sb[:], AF.Exp, bias=nmax_col[:], accum_out=sume[:]
            )
            ps_t2 = sps.tile([1, P], F32, tag="pst")
            nc.tensor.transpose(ps_t2[:], sume[:], ident_sb[:])
            row2 = smp.tile([1, P], F32, tag="row")
            nc.scalar.copy(row2[:], ps_t2[:])
            z = smp.tile([1, 1], F32, tag="z")
            nc.vector.tensor_reduce(
                z[:], row2[:], axis=mybir.AxisListType.X, op=mybir.AluOpType.add
            )
            rz = smp.tile([1, 1], F32, tag="rz")
            nc.vector.reciprocal(rz[:], z[:])
            zcol = smp.tile([P, 1], F32, tag="zcol")
            nc.vector.memset(zcol[:], 0.0)
            nc.vector.tensor_copy(zcol[0:1, :], rz[:])
            ps_b2 = sps.tile([P, 1], F32, tag="psb")
            nc.tensor.matmul(ps_b2[:], bc1f_sb[:], zcol[:], start=True, stop=True)
            rz_col = smp.tile([P, 1], F32, tag="rzcol")
            nc.scalar.copy(rz_col[:], ps_b2[:])

            attn_sb = smp.tile([P, ST], mm_dt, tag="attn")
            nc.vector.tensor_scalar_mul(attn_sb[:], expv[:], rz_col[:])
            if debug_mode in ("nosm", "nostride"):
                nc.sync.dma_start(
                    attn_out[b].rearrange("(p o) -> p o", p=P),
                    attn_sb[:].bitcast(mybir.dt.float32),
                )
            else:
                nc.sync.dma_start(
                    attn_out[b].rearrange("(o p) -> p o", p=P),
                    attn_sb[:].bitcast(mybir.dt.float32),
                )
            if debug_mode == "nosm":
                dbg_ctx = ctxp.tile([1, H], F32, tag=f"ctx{b}")
                nc.vector.tensor_copy(dbg_ctx[:], energy[0:1, :])
                nc.sync.dma_start(ctx_out[b : b + 1, :], dbg_ctx[:])
                continue

            # ---- context[h] = sum_s attn[s] * E[s,h] ----
            ctx_sb = ctxp.tile([1, H], F32, tag=f"ctx{b}")
            for n in range(NCH):
                pc = cps.tile([1, NF], F32, tag="pc")
                for t in range(ST):
                    nc.tensor.matmul(
                        pc[:],
                        attn_sb[:, t : t + 1],
                        en_sb[:, t, n * NF : (n + 1) * NF],
                        start=(t == 0),
                        stop=(t == ST - 1),
                    )
                nc.scalar.copy(ctx_sb[:, n * NF : (n + 1) * NF], pc[:])
            nc.sync.dma_start(ctx_out[b : b + 1, :], ctx_sb[:])
        if loop_cm is not None:
            loop_cm.__exit__(None, None, None)

    nc.compile()
    return nc


def _get_nc():
    if "nc" not in _CACHE:
        _CACHE["nc"] = _build()
    return _CACHE["nc"]


def _make_runner(nc):
    """Build a cached jitted SPMD runner for `nc` (one NEFF compile total).

    Mirrors concourse.bass2jax.run_bass_via_pjrt, but keeps the jitted
    callable so repeat invocations don't re-trace/re-compile.
    """
    import jax
    import numpy as jnp_np  # noqa: F401
    from jax.sharding import Mesh, PartitionSpec
    from jax.experimental.shard_map import shard_map

    import concourse.mybir as mybir
    from concourse import bass2jax

    bass2jax.install_neuronx_cc_hook()

    partition_name = nc.partition_id_tensor.name if nc.partition_id_tensor else None

    in_names, out_names, out_avals, zero_outs = [], [], [], []
    for alloc in nc.m.functions[0].allocations:
        if not isinstance(alloc, mybir.MemoryLocationSet):
            continue
        name = alloc.memorylocations[0].name
        if alloc.kind == "ExternalInput":
            if name != partition_name:
                in_names.append(name)
        elif alloc.kind == "ExternalOutput":
            out_names.append(name)
            shape = tuple(alloc.tensor_shape)
            dtype = mybir.dt.np(alloc.dtype)
            out_avals.append(jax.core.ShapedArray(shape, dtype))
            zero_outs.append(np.zeros(shape, dtype))
    n_params = len(in_names)
    all_names = in_names + out_names
    if partition_name is not None:
        all_names = all_names + [partition_name]

    def _body(*args):
        operands = list(args)
        if partition_name is not None:
            operands.append(bass2jax.partition_id_tensor())
        outs = bass2jax._bass_exec_p.bind(
            *operands,
            out_avals=tuple(out_avals),
            in_names=tuple(all_names),
            out_names=tuple(out_names),
            lowering_input_output_aliases=(),
            sim_require_finite=True,
            sim_require_nnan=True,
            nc=nc,
        )
        return tuple(outs)

    devices = jax.devices()[:NCORES]
    mesh = Mesh(np.asarray(devices), ("core",))
    n_args = n_params + len(out_names)
    sharded = jax.jit(
        shard_map(
            _body,
            mesh=mesh,
            in_specs=(PartitionSpec("core"),) * n_args,
            out_specs=(PartitionSpec("core"),) * len(out_names),
            check_rep=False,
        ),
        keep_unused=True,
    )

    def run(in_maps):
        concat_in = [
            np.concatenate([np.asarray(m[name]) for m in in_maps], axis=0)
            for name in in_names
        ]
        concat_zeros = [
            np.zeros((NCORES * z.shape[0], *z.shape[1:]), z.dtype) for z in zero_outs
        ]
        out_arrs = sharded(*concat_in, *concat_zeros)
        return [
            {
                name: np.asarray(out_arrs[i]).reshape(
                    NCORES, *out_avals[i].shape
                )[c]
                for i, name in enumerate(out_names)
            }
            for c in range(NCORES)
        ]

    run.sharded = sharded
    run.in_names = in_names
    run.out_names = out_names
    run.out_avals = out_avals
    run.zero_outs = zero_outs
    run.mesh = mesh
    return run


def _get_runner():
    if "runner" not in _CACHE:
        _CACHE["runner"] = _make_runner(_get_nc())
    return _CACHE["runner"]


def _prep_inputs(hidden, encoder_outputs, W, b, v):
    hidden = np.asarray(hidden, dtype=np.float32)
    enc = np.asarray(encoder_outputs, dtype=np.float32)
    W = np.asarray(W, dtype=np.float32)
    b = np.asarray(b, dtype=np.float32)
    v = np.asarray(v, dtype=np.float32)

    qb_all = (hidden[0] @ W[:, :H].T + b).astype(np.float32)  # [B, H]
    w2t_h = np.ascontiguousarray(W[:, H:].T)                  # [H, H]
    vb_h = np.ascontiguousarray(np.broadcast_to(v, (P, H)))
    bc1_h = np.zeros((P, P), np.float32)
    bc1_h[0, :] = 1.0
    ident_h = np.eye(P, dtype=np.float32)

    in_maps = []
    for c in range(NCORES):
        bs = slice(BL * c, BL * (c + 1))
        e_b = enc[:, bs, :]  # [S, BL, H]
        e_nat = np.ascontiguousarray(e_b.transpose(1, 0, 2))  # [BL, S, H]
        e_tt = np.ascontiguousarray(e_b.transpose(1, 2, 0))   # [BL, H, S]
        qbp = np.zeros((P, BL * H), np.float32)
        qbp[0, :] = qb_all[bs].reshape(-1)
        in_maps.append(
            dict(
                e_t=e_tt,
                e_nat=e_nat,
                w2t=w2t_h,
                qbp=qbp,
                vb=vb_h,
                bc1=bc1_h,
                bc1f=bc1_h,
                ident=ident_h,
            )
        )
    return in_maps


def kernel(hidden, encoder_outputs, W, b, v):
    in_maps = _prep_inputs(hidden, encoder_outputs, W, b, v)
    results = _get_runner()(in_maps)
    context = np.concatenate(
        [results[c]["ctx_out"] for c in range(NCORES)], axis=0
    ).astype(np.float32)
    attn = np.concatenate(
        [results[c]["attn_out"] for c in range(NCORES)], axis=0
    ).astype(np.float32)
    return context, attn
